# revision 2
# baseline (speedup 1.0000x reference)
"""Trainium2 Bass kernel for nn_DistancePredictor (pairwise MLP distance map).

out[b,i,j] = relu(W2 . gelu(cat(Xi,Xj,Xi-Xj,Xi*Xj) @ W1 + b1) + b2), symmetrized,
diagonal zeroed.  Decomposition (per row i):

    cat(...) @ W1 = X_j @ (Wp*X_i + (Wj-Wd)) + X_i @ (Wi+Wd)
                    `------- W_i (dxh) -----'   `--- A_i (bias) ---'

The row bias A_i + b1 is *seeded into PSUM by the PE* (a K=128 matmul of the
transposed-bias tile against a zero-stride broadcast identity column), so the
gelu needs no per-partition bias operand and one ACTIVATE can span 1.5 rows
(1536 cols = 3 PSUM banks).  That amortizes the ~185ns fixed ACT overhead per
instruction -- ACT is the bottleneck engine at ~94% busy.

Per 1536-col group: 3 bias-seed matmuls (fp16, start=True) + 3 fp32r S-matmuls
(accumulate, stop=True), one 1536-wide gelu (PSUM->SBUF fp16), then 12 128-col
x W2 matmuls that write the output *transposed* (j on partitions) into PSUM
accumulator banks.  Relu and the 0.5 symmetrize factor are folded into the
evacuation (W2,b2 pre-scaled by 0.5 on host).  The symmetrize term r'[j,i] is
fetched with a per-batch 8-core AllToAll of fp16 128x128 blocks (batch 0's
exchange overlaps batch 1's compute), transposed in-flight by the DMA xbar,
and added on GpSimd/DVE; the diagonal mask is per-core input data so the SPMD
program is identical on all cores.
"""

import numpy as np

import concourse.bacc as bacc
import concourse.mybir as mybir
import concourse.tile as tile
from concourse.bass_utils import run_bass_kernel_spmd

F32 = mybir.dt.float32
F32R = mybir.dt.float32r
F16 = mybir.dt.float16
AF = mybir.ActivationFunctionType
ALU = mybir.AluOpType

B, L, D = 2, 1024, 128
H = 128
NCORES = 8
SLAB = L // NCORES  # 128
NCHUNK = 2 * SLAB  # 512-col chunks per batch (2 per row)
NGROUP = (NCHUNK + 2) // 3  # 86: 85 full 1536-col groups + one 512-col tail


def build_nc(skip_collective=False, reps=1):
    nc = bacc.Bacc(
        "TRN2",
        target_bir_lowering=False,
        debug=False,
        num_devices=NCORES,
    )

    xt_in = nc.dram_tensor("xt", [B, D, L], F32R, kind="ExternalInput")
    xc_in = nc.dram_tensor("xc", [B, D, SLAB], F32, kind="ExternalInput")
    wp_in = nc.dram_tensor("wp", [D, H], F32, kind="ExternalInput")
    wb_in = nc.dram_tensor("wb", [D, H], F32, kind="ExternalInput")
    wa_in = nc.dram_tensor("wa", [D, H], F32, kind="ExternalInput")
    w2_in = nc.dram_tensor("w2h", [H, 1], F16, kind="ExternalInput")
    b1_in = nc.dram_tensor("b1r", [128, H], F32, kind="ExternalInput")
    b2_in = nc.dram_tensor("b2c", [128, 1], F32, kind="ExternalInput")
    eye_in = nc.dram_tensor("eye", [128, 128], F16, kind="ExternalInput")
    masks_in = nc.dram_tensor("masks", [128, NCORES * 128], F16, kind="ExternalInput")
    out_t = nc.dram_tensor("out", [B, L, SLAB], F16, kind="ExternalOutput")

    with tile.TileContext(nc) as tc:
        with (
            tc.tile_pool(name="const", bufs=1) as cp,
            tc.tile_pool(name="wpool", bufs=6) as wp_pool,
            tc.tile_pool(name="gpool", bufs=4) as g_pool,
            tc.tile_pool(name="rt", bufs=1) as rt_pool,
            tc.tile_pool(name="fin", bufs=8) as fin_pool,
            tc.tile_pool(name="ps_s", bufs=2, space="PSUM") as ps_s,
            tc.tile_pool(name="ps_acc", bufs=1, space="PSUM") as ps_acc,
            tc.tile_pool(name="dram", bufs=1, space="DRAM") as dram_pool,
        ):
            # ---- load constants / inputs to SBUF, spread across DMA queues
            # so the first group's dependency chain resolves fast ----
            xc_sb = [cp.tile([D, SLAB], F32, name=f"xc_sb{b}") for b in range(B)]
            wp_sb = cp.tile([D, H], F32, name="wp_sb")
            wb_sb = cp.tile([D, H], F32, name="wb_sb")
            wa_sb = cp.tile([D, H], F32, name="wa_sb")
            b1_sb = cp.tile([128, H], F32, name="b1_sb")
            eye_sb = cp.tile([128, 128], F16, name="eye_sb")
            w2_sb = cp.tile([H, 1], F16, name="w2_sb")
            b2_sb = cp.tile([128, 1], F32, name="b2_sb")
            xt_sb = [cp.tile([D, L], F32R, name=f"xt_sb{b}") for b in range(B)]
            masks_sb = cp.tile([128, NCORES * 128], F16, name="masks_sb")

            # sync queue: batch-0 critical path
            nc.sync.dma_start(xc_sb[0][:], xc_in[0])
            nc.sync.dma_start(xt_sb[0][:, 0:512], xt_in[0][:, 0:512])
            nc.sync.dma_start(xt_sb[0][:, 512:1024], xt_in[0][:, 512:1024])
            # scalar (ACT hwdge, free until first gelu): bias/weight path
            nc.scalar.dma_start(wa_sb[:], wa_in[:])
            nc.scalar.dma_start(wp_sb[:], wp_in[:])
            nc.scalar.dma_start(wb_sb[:], wb_in[:])
            nc.scalar.dma_start(eye_sb[:], eye_in[:])
            nc.scalar.dma_start(w2_sb[:], w2_in[:])
            # gpsimd (swdge): everything else / batch 1
            nc.gpsimd.dma_start(b1_sb[:], b1_in[:])
            nc.gpsimd.dma_start(xc_sb[1][:], xc_in[1])
            nc.gpsimd.dma_start(b2_sb[:], b2_in[:])
            nc.gpsimd.dma_start(xt_sb[1][:, 0:512], xt_in[1][:, 0:512])
            nc.gpsimd.dma_start(xt_sb[1][:, 512:1024], xt_in[1][:, 512:1024])
            nc.gpsimd.dma_start(masks_sb[:], masks_in[:])

            # Preload the gelu activation-table set (~2.7us) while inputs
            # stream in, instead of stalling the first real gelu on it.
            warm = cp.tile([128, 1], F32, name="warm")
            nc.scalar.activation(warm[:], wp_sb[:, 0:1], AF.Gelu, bias=0.0, scale=1.0)

            # ---- atT[il, h] = Xc^T Wa + b1 (transposed bias, fp16) ----
            # Seeds read it as a K=128 stationary; eye-column broadcast picks
            # the row.
            at_sb = []
            for b in range(B):
                at_ps = ps_s.tile([128, 1536], F32, tag="s", name=f"at_ps{b}")
                nc.tensor.matmul(
                    at_ps[:, 0:H], xc_sb[b][:], wa_sb[:], start=True, stop=True
                )
                atb = cp.tile([SLAB, H], F16, name=f"at_sb{b}")
                nc.vector.tensor_tensor(
                    atb[:], at_ps[:, 0:H], b1_sb[:], op=ALU.add
                )
                at_sb.append(atb)

            # ---- A2A buffers in DRAM (per batch, so batch 0's exchange +
            # symmetrize overlap batch 1's compute) ----
            a2a_send = [
                dram_pool.tile([NCORES, SLAB, SLAB], F16, name=f"a2a_send{b}")
                for b in range(B)
            ]
            a2a_recv = [
                dram_pool.tile([NCORES, SLAB, SLAB], F16, name=f"a2a_recv{b}")
                for b in range(B)
            ]

            # ---- main loop: 86 groups of (up to) three 512-col chunks ----
            rt_tiles = {}
            for rep, b in [(r, b) for r in range(reps) for b in range(B)]:
                acc0 = ps_acc.tile(
                    [128, 4 * SLAB], F32, tag="acc0", name=f"acc0_{rep}_{b}"
                )
                acc1 = ps_acc.tile(
                    [128, 4 * SLAB], F32, tag="acc1", name=f"acc1_{rep}_{b}"
                )
                accs = [acc0, acc1]
                xtr = xt_sb[b][:]
                wtiles = {}
                for g in range(NGROUP):
                    chunks = list(range(3 * g, min(3 * g + 3, NCHUNK)))
                    glen = 512 * len(chunks)
                    ps = ps_s.tile([128, 1536], F32, tag="s", name=f"ps_{b}_{g}")
                    # bias seeds: ps[:, c] = atT[row]  (broadcast identity col)
                    for ci, C in enumerate(chunks):
                        row = C // 2
                        nc.tensor.matmul(
                            ps[:, ci * 512 : (ci + 1) * 512],
                            at_sb[b][:],
                            eye_sb[:, row : row + 1].broadcast_to((128, 512)),
                            start=True,
                            stop=False,
                        )
                    # S accumulate: ps[:, c] += W_row^T @ X[:, cols]
                    for ci, C in enumerate(chunks):
                        row, xc0 = C // 2, (C % 2) * 512
                        if row not in wtiles:
                            wt = wp_pool.tile([D, H], F32R, tag="wi")
                            nc.vector.scalar_tensor_tensor(
                                wt[:],
                                wp_sb[:],
                                xc_sb[b][:, row : row + 1],
                                wb_sb[:],
                                op0=ALU.mult,
                                op1=ALU.add,
                            )
                            wtiles = {row: wt}  # keep only the latest
                        nc.tensor.matmul(
                            ps[:, ci * 512 : (ci + 1) * 512],
                            wtiles[row][:],
                            xtr[:, xc0 : xc0 + 512],
                            start=False,
                            stop=True,
                        )
                    gt = g_pool.tile([128, 1536], F16, tag="g")
                    nc.scalar.activation(
                        gt[:, 0:glen], ps[:, 0:glen], AF.Gelu, bias=0.0, scale=1.0
                    )
                    for s in range(glen // 128):
                        flat = g * 1536 + s * 128
                        il = flat // 1024
                        jt = (flat % 1024) // 128
                        q, sub = jt // 4, jt % 4
                        col = sub * SLAB + il
                        nc.tensor.matmul(
                            accs[q][:, col : col + 1],
                            gt[:, s * 128 : (s + 1) * 128],
                            w2_sb[:],
                            start=True,
                            stop=True,
                        )
                # evacuate accumulators: relu(x + b2/2) -> sbuf (fp16), stage
                # this q-half to the A2A send buffer in one chunked DMA.
                # On DVE (fused add+max) to keep ACT free for gelus.
                last_b = rep == reps - 1 and b == B - 1
                for q in range(2):
                    rt = rt_pool.tile([128, 4 * SLAB], F16, name=f"rt_{b}_{q}")
                    if last_b and q == 1:
                        # ACT is idle after the final gelu; run this half
                        # there so both evacuations go in parallel.
                        nc.scalar.activation(
                            rt[:], accs[q][:], AF.Relu, bias=b2_sb[:], scale=1.0
                        )
                    else:
                        nc.vector.tensor_scalar(
                            rt[:],
                            accs[q][:],
                            b2_sb[:],
                            0.0,
                            op0=ALU.add,
                            op1=ALU.max,
                        )
                    # Zero this core's diagonal block BEFORE staging: the
                    # masked values then come back from the AllToAll already
                    # masked, so no per-block mask pass is needed later.
                    mw = nc.vector if last_b else nc.gpsimd
                    mw.tensor_tensor(
                        rt[:], rt[:], masks_sb[:, q * 512 : (q + 1) * 512],
                        op=ALU.mult,
                    )
                    rt_tiles[(b, q)] = rt
                    # ACT's HWDGE queue is free once the final batch's gelus
                    # are done; before that it would stall gelus (engine FIFO)
                    stage_eng = nc.sync if q == 0 else (
                        nc.scalar if last_b else nc.gpsimd
                    )
                    stage_eng.dma_start(
                        a2a_send[b][4 * q : 4 * q + 4].rearrange("s r c -> r s c"),
                        rt[:].rearrange("r (s c) -> r s c", s=4),
                    )

                # all-to-all this batch's transposed-slab blocks
                if not skip_collective:
                    nc.gpsimd.collective_compute(
                        "AllToAll",
                        ALU.bypass,
                        replica_groups=[list(range(NCORES))],
                        ins=[a2a_send[b].opt()],
                        outs=[a2a_recv[b].opt()],
                    )

                # symmetrize: out[b, d-block, :] = own + recv^T (the diag
                # mask was already applied to rt pre-exchange; recv blocks are
                # transposed in-flight by the DMA xbar)
                for d in range(NCORES):
                    rbt = fin_pool.tile([128, 128], F16, tag="rbt")
                    if last_b and d % 2 == 0:
                        nc.scalar.dma_start_transpose(rbt[:], a2a_recv[b][d])
                    else:
                        nc.sync.dma_start_transpose(rbt[:], a2a_recv[b][d])
                    q, sub = d // 4, d % 4
                    own = rt_tiles[(b, q)][:, sub * SLAB : (sub + 1) * SLAB]
                    ob = fin_pool.tile([128, 128], F16, tag="ob")
                    # During earlier batches DVE is busy with W_i preps and a
                    # queued epilogue op would stall them (engine FIFO), so do
                    # the adds on GpSimd; on the final batch DVE is free.
                    if last_b:
                        ew = nc.vector if d % 4 != 3 else nc.gpsimd
                    else:
                        ew = nc.gpsimd
                    ew.tensor_tensor(ob[:], rbt[:], own, op=ALU.add)
                    if last_b:
                        store_eng = nc.sync if d % 2 == 1 else nc.scalar
                    else:
                        store_eng = nc.sync if d % 2 == 1 else nc.gpsimd
                    store_eng.dma_start(out_t[b, d * 128 : (d + 1) * 128, :], ob[:])

    nc.compile()
    return nc


_NC_CACHE = {}


def _get_nc():
    if "nc" not in _NC_CACHE:
        _NC_CACHE["nc"] = build_nc()
    return _NC_CACHE["nc"]


def make_in_maps(X, W1, b1, W2, b2):
    X = np.ascontiguousarray(X, dtype=np.float32)
    W1 = np.asarray(W1, dtype=np.float32)
    b1 = np.asarray(b1, dtype=np.float32)
    W2 = np.asarray(W2, dtype=np.float32)
    b2 = np.asarray(b2, dtype=np.float32)

    Wi, Wj, Wd, Wp = W1[0:128], W1[128:256], W1[256:384], W1[384:512]
    wa = np.ascontiguousarray(Wi + Wd)
    wb = np.ascontiguousarray(Wj - Wd)
    wp = np.ascontiguousarray(Wp)
    w2h = np.ascontiguousarray((0.5 * W2).astype(np.float16).reshape(H, 1))
    b1r = np.ascontiguousarray(np.tile(b1.reshape(1, H), (128, 1)))
    b2c = np.full((128, 1), 0.5 * float(b2[0]), dtype=np.float32)
    eye = np.eye(128, dtype=np.float16)
    xt = np.ascontiguousarray(X.transpose(0, 2, 1))  # (B, D, L)

    in_maps = []
    for c in range(NCORES):
        masks = np.ones((128, NCORES * 128), dtype=np.float16)
        masks[:, c * 128 : (c + 1) * 128] = (
            1.0 - np.eye(128)
        ).astype(np.float16)
        xc = np.ascontiguousarray(xt[:, :, c * SLAB : (c + 1) * SLAB])
        in_maps.append(
            {
                "xt": xt,
                "xc": xc,
                "wp": wp,
                "wb": wb,
                "wa": wa,
                "w2h": w2h,
                "b1r": b1r,
                "b2c": b2c,
                "eye": eye,
                "masks": masks,
            }
        )
    return in_maps


def assemble(results):
    full = np.empty((B, L, L), dtype=np.float32)
    for c in range(NCORES):
        o = results[c]["out"]  # (B, L, SLAB) fp16: out[b, j, i_local]
        full[:, c * SLAB : (c + 1) * SLAB, :] = o.transpose(0, 2, 1).astype(
            np.float32
        )
    return full


def kernel(X, W1, b1, W2, b2, _trace=False):
    nc = _get_nc()
    in_maps = make_in_maps(X, W1, b1, W2, b2)
    res = run_bass_kernel_spmd(
        nc, in_maps, core_ids=list(range(NCORES)), trace=_trace
    )
    out = assemble(res.results)
    if _trace:
        return out, res
    return out


if __name__ == "__main__":
    nc = build_nc()
    print("compiled ok")


# revision 6
# speedup vs baseline: 1.0763x; 1.0763x over previous
"""Trainium2 Bass kernel for nn_DistancePredictor (pairwise MLP distance map).

out[b,i,j] = relu(W2 . gelu(cat(Xi,Xj,Xi-Xj,Xi*Xj) @ W1 + b1) + b2), symmetrized,
diagonal zeroed.  Decomposition (per row i):

    cat(...) @ W1 = X_j @ (Wp*X_i + (Wj-Wd)) + X_i @ (Wi+Wd)
                    `------- W_i (dxh) -----'   `--- A_i (bias) ---'

The row bias A_i + b1 is *seeded into PSUM by the PE* (a K=128 matmul of the
transposed-bias tile against a zero-stride broadcast identity column), so the
gelu needs no per-partition bias operand and one ACTIVATE can span 1.5 rows
(1536 cols = 3 PSUM banks).  That amortizes the ~185ns fixed ACT overhead per
instruction -- ACT is the bottleneck engine at ~94% busy.

Per 1536-col group: 3 bias-seed matmuls (fp16, start=True) + 3 fp32r S-matmuls
(accumulate, stop=True), one 1536-wide gelu (PSUM->SBUF fp16), then 12 128-col
x W2 matmuls that write the output *transposed* (j on partitions) into PSUM
accumulator banks.  Relu and the 0.5 symmetrize factor are folded into the
evacuation (W2,b2 pre-scaled by 0.5 on host).  The symmetrize term r'[j,i] is
fetched with a per-batch 8-core AllToAll of fp16 128x128 blocks (batch 0's
exchange overlaps batch 1's compute), transposed in-flight by the DMA xbar,
and added on GpSimd/DVE; the diagonal mask is per-core input data so the SPMD
program is identical on all cores.
"""

import numpy as np

import concourse.bacc as bacc
import concourse.mybir as mybir
import concourse.tile as tile
from concourse.bass_utils import run_bass_kernel_spmd

F32 = mybir.dt.float32
F32R = mybir.dt.float32r
F16 = mybir.dt.float16
AF = mybir.ActivationFunctionType
ALU = mybir.AluOpType

B, L, D = 2, 1024, 128
H = 128
NCORES = 8
SLAB = L // NCORES  # 128
NCHUNK = 2 * SLAB  # 512-col chunks per batch (2 per row)
NGROUP = (NCHUNK + 2) // 3  # 86: 85 full 1536-col groups + one 512-col tail


def build_nc(skip_collective=False, reps=1):
    nc = bacc.Bacc(
        "TRN2",
        target_bir_lowering=False,
        debug=False,
        num_devices=NCORES,
    )

    xt_in = nc.dram_tensor("xt", [B, D, L], F32R, kind="ExternalInput")
    xc_in = nc.dram_tensor("xc", [B, D, SLAB], F32, kind="ExternalInput")
    wp_in = nc.dram_tensor("wp", [D, H], F32, kind="ExternalInput")
    wb_in = nc.dram_tensor("wb", [D, H], F32, kind="ExternalInput")
    wa_in = nc.dram_tensor("wa", [D, H], F32, kind="ExternalInput")
    w2_in = nc.dram_tensor("w2h", [H, 1], F16, kind="ExternalInput")
    b1_in = nc.dram_tensor("b1r", [128, H], F32, kind="ExternalInput")
    b2_in = nc.dram_tensor("b2c", [128, 1], F32, kind="ExternalInput")
    eye_in = nc.dram_tensor("eye", [128, 128], F16, kind="ExternalInput")
    masks_in = nc.dram_tensor("masks", [128, NCORES * 128], F16, kind="ExternalInput")
    out_t = nc.dram_tensor("out", [B, L, SLAB], F16, kind="ExternalOutput")

    with tile.TileContext(nc) as tc:
        with (
            tc.tile_pool(name="const", bufs=1) as cp,
            tc.tile_pool(name="wpool", bufs=6) as wp_pool,
            tc.tile_pool(name="gpool", bufs=4) as g_pool,
            tc.tile_pool(name="rt", bufs=1) as rt_pool,
            tc.tile_pool(name="fin", bufs=8) as fin_pool,
            tc.tile_pool(name="ps_s", bufs=1, space="PSUM") as ps_s,
            tc.tile_pool(name="ps_acc", bufs=1, space="PSUM") as ps_acc,
            tc.tile_pool(name="dram", bufs=1, space="DRAM") as dram_pool,
        ):
            # ---- load constants / inputs to SBUF, spread across DMA queues
            # so the first group's dependency chain resolves fast ----
            xc_sb = [cp.tile([D, SLAB], F32, name=f"xc_sb{b}") for b in range(B)]
            wp_sb = cp.tile([D, H], F32, name="wp_sb")
            wb_sb = cp.tile([D, H], F32, name="wb_sb")
            wa_sb = cp.tile([D, H], F32, name="wa_sb")
            b1_sb = cp.tile([128, H], F32, name="b1_sb")
            eye_sb = cp.tile([128, 128], F16, name="eye_sb")
            w2_sb = cp.tile([H, 1], F16, name="w2_sb")
            b2_sb = cp.tile([128, 1], F32, name="b2_sb")
            xt_sb = [cp.tile([D, L], F32R, name=f"xt_sb{b}") for b in range(B)]
            masks_sb = cp.tile([128, NCORES * 128], F16, name="masks_sb")

            # sync queue: batch-0 critical path
            nc.sync.dma_start(xc_sb[0][:], xc_in[0])
            nc.sync.dma_start(xt_sb[0][:, 0:512], xt_in[0][:, 0:512])
            nc.sync.dma_start(xt_sb[0][:, 512:1024], xt_in[0][:, 512:1024])
            # scalar (ACT hwdge, free until first gelu): bias/weight path
            nc.scalar.dma_start(wa_sb[:], wa_in[:])
            nc.scalar.dma_start(wp_sb[:], wp_in[:])
            nc.scalar.dma_start(wb_sb[:], wb_in[:])
            nc.scalar.dma_start(eye_sb[:], eye_in[:])
            nc.scalar.dma_start(w2_sb[:], w2_in[:])
            # gpsimd (swdge): everything else / batch 1
            nc.gpsimd.dma_start(b1_sb[:], b1_in[:])
            nc.gpsimd.dma_start(xc_sb[1][:], xc_in[1])
            nc.gpsimd.dma_start(b2_sb[:], b2_in[:])
            nc.gpsimd.dma_start(xt_sb[1][:, 0:512], xt_in[1][:, 0:512])
            nc.gpsimd.dma_start(xt_sb[1][:, 512:1024], xt_in[1][:, 512:1024])
            nc.gpsimd.dma_start(masks_sb[:], masks_in[:])

            # Preload the gelu activation-table set (~2.7us) while inputs
            # stream in, instead of stalling the first real gelu on it.
            warm = cp.tile([128, 1], F32, name="warm")
            nc.scalar.activation(warm[:], wp_sb[:, 0:1], AF.Gelu, bias=0.0, scale=1.0)

            # ---- atT[il, h] = Xc^T Wa + b1 (transposed bias, fp16) ----
            # Seeds read it as a K=128 stationary; eye-column broadcast picks
            # the row.
            at_sb = []
            for b in range(B):
                at_ps = ps_s.tile(
                    [128, 1536], F32, tag="sA" if b == 0 else "sB", name=f"at_ps{b}"
                )
                nc.tensor.matmul(
                    at_ps[:, 0:H], xc_sb[b][:], wa_sb[:], start=True, stop=True
                )
                atb = cp.tile([SLAB, H], F16, name=f"at_sb{b}")
                nc.vector.tensor_tensor(
                    atb[:], at_ps[:, 0:H], b1_sb[:], op=ALU.add
                )
                at_sb.append(atb)

            # ---- A2A buffers in DRAM (per batch, so batch 0's exchange +
            # symmetrize overlap batch 1's compute) ----
            a2a_send = [
                dram_pool.tile([NCORES, SLAB, SLAB], F16, name=f"a2a_send{b}")
                for b in range(B)
            ]
            a2a_recv = [
                dram_pool.tile([NCORES, SLAB, SLAB], F16, name=f"a2a_recv{b}")
                for b in range(B)
            ]

            # ---- main loop ----
            # Chunks (512 cols each) are dealt to three rotating PSUM
            # buffers -- 1536/1536/512 cols -- so one gelu can span 1.5 rows
            # while the other two buffers refill; the W2 accumulators for
            # both j-halves share a single PSUM bank (il in 64-row phases,
            # evacuated twice per batch) to make the third buffer fit.
            rt_tiles = {}
            for rep, b in [(r, b) for r in range(reps) for b in range(B)]:
                xtr = xt_sb[b][:]
                wtiles = {}
                accs = None
                cur_phase = -1
                last_b = rep == reps - 1 and b == B - 1

                def emit_evac(ph, accs_, b=b, last_b=last_b):
                    for q in range(2):
                        if ph == 0:
                            rt_tiles[(b, q)] = rt_pool.tile(
                                [128, 4 * SLAB], F16, name=f"rt_{b}_{q}"
                            )
                        rt = rt_tiles[(b, q)]
                        dst = rt[:].rearrange("r (s c) -> r s c", s=4)[
                            :, :, ph * 64 : (ph + 1) * 64
                        ]
                        src = accs_[0][:, q * 256 : (q + 1) * 256].rearrange(
                            "r (s c) -> r s c", s=4
                        )
                        if last_b and ph == 1 and q == 1:
                            # ACT is idle after the final gelu; run this one
                            # there so both final evacuations go in parallel.
                            nc.scalar.activation(
                                dst, src, AF.Relu, bias=b2_sb[:], scale=1.0
                            )
                        else:
                            nc.vector.tensor_scalar(
                                dst, src, b2_sb[:], 0.0, op0=ALU.add, op1=ALU.max
                            )

                groups = []
                c = 0
                cyc = [("sA", 3), ("sB", 3), ("sC", 1)]
                gi = 0
                while c < NCHUNK:
                    tag, n = cyc[gi % 3]
                    gi += 1
                    n = min(n, NCHUNK - c)
                    groups.append((tag, list(range(c, c + n))))
                    c += n

                for tag, chunks in groups:
                    glen = 512 * len(chunks)
                    width = 512 if tag == "sC" else 1536
                    ps = ps_s.tile([128, width], F32, tag=tag)
                    # bias seeds: ps[:, c] = atT[row] (broadcast identity col)
                    for ci, C in enumerate(chunks):
                        row = C // 2
                        nc.tensor.matmul(
                            ps[:, ci * 512 : (ci + 1) * 512],
                            at_sb[b][:],
                            eye_sb[:, row : row + 1].broadcast_to((128, 512)),
                            start=True,
                            stop=False,
                        )
                    # S accumulate: ps[:, c] += W_row^T @ X[:, cols]
                    for ci, C in enumerate(chunks):
                        row, xcol = C // 2, (C % 2) * 512
                        if row not in wtiles:
                            wt = wp_pool.tile([D, H], F32R, tag="wi")
                            nc.vector.scalar_tensor_tensor(
                                wt[:],
                                wp_sb[:],
                                xc_sb[b][:, row : row + 1],
                                wb_sb[:],
                                op0=ALU.mult,
                                op1=ALU.add,
                            )
                            wtiles = {row: wt}  # keep only the latest
                        nc.tensor.matmul(
                            ps[:, ci * 512 : (ci + 1) * 512],
                            wtiles[row][:],
                            xtr[:, xcol : xcol + 512],
                            start=False,
                            stop=True,
                        )
                    gt = g_pool.tile([128, 1536], F16, tag="g")
                    nc.scalar.activation(
                        gt[:, 0:glen], ps[:, 0:glen], AF.Gelu, bias=0.0, scale=1.0
                    )
                    for ci, C in enumerate(chunks):
                        il = C // 2
                        phase = il // 64
                        if phase != cur_phase:
                            if accs is not None:
                                emit_evac(cur_phase, accs)
                            accs = [
                                ps_acc.tile(
                                    [128, 512],
                                    F32,
                                    tag="acc",
                                    name=f"acc_{rep}_{b}_{phase}",
                                )
                            ]
                            cur_phase = phase
                        q = C % 2
                        for s in range(4):
                            sub = s  # jt % 4
                            col = q * 256 + sub * 64 + (il - 64 * phase)
                            nc.tensor.matmul(
                                accs[0][:, col : col + 1],
                                gt[:, (ci * 4 + s) * 128 : (ci * 4 + s + 1) * 128],
                                w2_sb[:],
                                start=True,
                                stop=True,
                            )
                emit_evac(cur_phase, accs)

                # mask + stage: zero this core's diagonal block BEFORE
                # staging so the values come back from the AllToAll already
                # masked; then one chunked DMA per q-half to the A2A send
                # buffer.
                for q in range(2):
                    rt = rt_tiles[(b, q)]
                    mw = nc.vector if last_b else nc.gpsimd
                    mw.tensor_tensor(
                        rt[:], rt[:], masks_sb[:, q * 512 : (q + 1) * 512],
                        op=ALU.mult,
                    )
                    # ACT's HWDGE queue is free once the final batch's gelus
                    # are done; before that it would stall gelus (engine FIFO)
                    stage_eng = nc.sync if q == 0 else (
                        nc.scalar if last_b else nc.gpsimd
                    )
                    stage_eng.dma_start(
                        a2a_send[b][4 * q : 4 * q + 4].rearrange("s r c -> r s c"),
                        rt[:].rearrange("r (s c) -> r s c", s=4),
                    )

                # all-to-all this batch's transposed-slab blocks
                if not skip_collective:
                    nc.gpsimd.collective_compute(
                        "AllToAll",
                        ALU.bypass,
                        replica_groups=[list(range(NCORES))],
                        ins=[a2a_send[b].opt()],
                        outs=[a2a_recv[b].opt()],
                    )

                # symmetrize: out[b, d-block, :] = own + recv^T (the diag
                # mask was already applied to rt pre-exchange; recv blocks are
                # transposed in-flight by the DMA xbar)
                for d in range(NCORES):
                    rbt = fin_pool.tile([128, 128], F16, tag="rbt")
                    if last_b and d % 2 == 0:
                        nc.scalar.dma_start_transpose(rbt[:], a2a_recv[b][d])
                    else:
                        nc.sync.dma_start_transpose(rbt[:], a2a_recv[b][d])
                    q, sub = d // 4, d % 4
                    own = rt_tiles[(b, q)][:, sub * SLAB : (sub + 1) * SLAB]
                    ob = fin_pool.tile([128, 128], F16, tag="ob")
                    # During earlier batches DVE is busy with W_i preps and a
                    # queued epilogue op would stall them (engine FIFO), so do
                    # the adds on GpSimd; on the final batch DVE is free.
                    if last_b:
                        ew = nc.vector if d % 4 != 3 else nc.gpsimd
                    else:
                        ew = nc.gpsimd
                    ew.tensor_tensor(ob[:], rbt[:], own, op=ALU.add)
                    if last_b:
                        store_eng = nc.sync if d % 2 == 1 else nc.scalar
                    else:
                        store_eng = nc.sync if d % 2 == 1 else nc.gpsimd
                    store_eng.dma_start(out_t[b, d * 128 : (d + 1) * 128, :], ob[:])

    nc.compile()
    return nc


_NC_CACHE = {}


def _get_nc():
    if "nc" not in _NC_CACHE:
        _NC_CACHE["nc"] = build_nc()
    return _NC_CACHE["nc"]


def make_in_maps(X, W1, b1, W2, b2):
    X = np.ascontiguousarray(X, dtype=np.float32)
    W1 = np.asarray(W1, dtype=np.float32)
    b1 = np.asarray(b1, dtype=np.float32)
    W2 = np.asarray(W2, dtype=np.float32)
    b2 = np.asarray(b2, dtype=np.float32)

    Wi, Wj, Wd, Wp = W1[0:128], W1[128:256], W1[256:384], W1[384:512]
    wa = np.ascontiguousarray(Wi + Wd)
    wb = np.ascontiguousarray(Wj - Wd)
    wp = np.ascontiguousarray(Wp)
    w2h = np.ascontiguousarray((0.5 * W2).astype(np.float16).reshape(H, 1))
    b1r = np.ascontiguousarray(np.tile(b1.reshape(1, H), (128, 1)))
    b2c = np.full((128, 1), 0.5 * float(b2[0]), dtype=np.float32)
    eye = np.eye(128, dtype=np.float16)
    xt = np.ascontiguousarray(X.transpose(0, 2, 1))  # (B, D, L)

    in_maps = []
    for c in range(NCORES):
        masks = np.ones((128, NCORES * 128), dtype=np.float16)
        masks[:, c * 128 : (c + 1) * 128] = (
            1.0 - np.eye(128)
        ).astype(np.float16)
        xc = np.ascontiguousarray(xt[:, :, c * SLAB : (c + 1) * SLAB])
        in_maps.append(
            {
                "xt": xt,
                "xc": xc,
                "wp": wp,
                "wb": wb,
                "wa": wa,
                "w2h": w2h,
                "b1r": b1r,
                "b2c": b2c,
                "eye": eye,
                "masks": masks,
            }
        )
    return in_maps


def assemble(results):
    full = np.empty((B, L, L), dtype=np.float32)
    for c in range(NCORES):
        o = results[c]["out"]  # (B, L, SLAB) fp16: out[b, j, i_local]
        full[:, c * SLAB : (c + 1) * SLAB, :] = o.transpose(0, 2, 1).astype(
            np.float32
        )
    return full


def kernel(X, W1, b1, W2, b2, _trace=False):
    nc = _get_nc()
    in_maps = make_in_maps(X, W1, b1, W2, b2)
    res = run_bass_kernel_spmd(
        nc, in_maps, core_ids=list(range(NCORES)), trace=_trace
    )
    out = assemble(res.results)
    if _trace:
        return out, res
    return out


if __name__ == "__main__":
    nc = build_nc()
    print("compiled ok")
